# Initial kernel scaffold
#
"""Trainium2 Bass kernel for nn_Net_48498770706963 (retrieval_knn).

Computation (see reference):
  emb   = sum_t emb_table[x[n, t]]          # embedding-bag over T=100 tokens
  query = relu(emb / ||emb||_2 + bias)      # [N, D]
  out   = query @ W[sample_ids].T + b_cls[sample_ids]   # [N, S]

Distribution: data-parallel over the batch. Each of the 8 NeuronCores
computes the embedding-bag + normalization for its 128 samples and the full
sampled-logits panel [S, 128] for those samples (class weight table
replicated). No collectives. Host work is limited to slicing/padding/
transposing inputs and reassembling the output.

Per-core kernel pipeline (all validated on silicon):
  - 128 indirect DMA gathers [100, 128] (one per sample) feed per-sample
    ones-matmuls that reduce tokens on the TensorEngine directly into the
    transposed query layout qT [D, 128].
  - L2-normalization in transposed layout: DVE square, ones-matmul column
    sum, ACT sqrt + DVE reciprocal, ones-matmul partition broadcast, DVE
    scale, ACT relu with per-partition bias.
  - 157 indirect DMA gathers of [128, 129] rows from the packed [W | b_cls]
    table, PE transpose of the W block, logits matmul against qT, DVE
    bias-add (broadcast of the gathered bias column) fused with the
    PSUM->SBUF copy, direct DMA store of the [128, 128] output tile.
"""

import numpy as np

import concourse.bass as bass
import concourse.mybir as mybir
from concourse.tile import TileContext
from concourse.masks import make_identity

N, T, D = 1024, 100, 128
S = 20000
V_IN = 135909
V_OUT = 670091
N_CORES = 8
P = 128
NB = N // N_CORES            # 128 samples per core
S_TILES = (S + P - 1) // P   # 157
S_PAD = S_TILES * P          # 20096

f32 = mybir.dt.float32
i32 = mybir.dt.int32

_MAX_WAITS = 1


def _fix_sync_waits(nc):
    """This walrus build rejects instructions carrying more than one sem
    wait ('Too many sync wait commands'). Hoist excess waits onto NoOps
    inserted immediately before, on the same engine stream."""
    for fn in nc.m.functions:
        for bb in fn.blocks:
            out = []
            changed = False
            for inst in bb.instructions:
                si = inst.sync_info
                waits = list(si.on_wait) if si is not None else []
                if len(waits) > _MAX_WAITS:
                    changed = True
                    excess, keep = waits[:-_MAX_WAITS], waits[-_MAX_WAITS:]
                    for k in range(0, len(excess), _MAX_WAITS):
                        nop = mybir.InstNoOp(
                            name=nc.get_next_instruction_name(), ins=[], outs=[]
                        )
                        nop.engine = inst.engine
                        nop.sync_info = mybir.SyncInfo(
                            on_wait=excess[k : k + _MAX_WAITS], on_update=[]
                        )
                        out.append(nop)
                    si.on_wait = keep
                out.append(inst)
            if changed:
                bb.instructions = out


def build_nc(iters: int = 1):
    """Build the per-core Bass program. iters>1 wraps the body in a For_i
    loop (used only for wall-clock benchmarking in test.py)."""
    nc = bass.Bass()
    xT_d = nc.declare_dram_parameter("xT", [T, NB], i32, isOutput=False)
    emb_d = nc.declare_dram_parameter(
        "emb_table", [V_IN + 1, D], f32, isOutput=False
    )
    bias_d = nc.declare_dram_parameter("bias", [D], f32, isOutput=False)
    Wb_d = nc.declare_dram_parameter("Wb", [V_OUT, D + 1], f32, isOutput=False)
    sidT_d = nc.declare_dram_parameter("sidT", [P, S_TILES], i32, isOutput=False)
    out_d = nc.declare_dram_parameter("out", [S_PAD, NB], f32, isOutput=True)

    with TileContext(nc) as tc:
        with (
            tc.tile_pool(name="const", bufs=1) as constp,
            tc.tile_pool(name="ebuf", bufs=8) as ebuf,
            tc.tile_pool(name="nbuf", bufs=2) as nbuf,
            tc.tile_pool(name="psum1", bufs=1, space="PSUM") as psum1,
            tc.tile_pool(name="psum2", bufs=2, space="PSUM") as psum2,
            tc.tile_pool(name="wpool", bufs=6) as wpool,
            tc.tile_pool(name="opool", bufs=6) as opool,
        ):
            # ---- constants ----
            identity = constp.tile([P, P], f32)
            make_identity(nc, identity[:])
            ones_col = constp.tile([P, 1], f32)
            nc.vector.memset(ones_col[:], 1.0)
            ones_row = constp.tile([1, P], f32)
            nc.vector.memset(ones_row[:], 1.0)
            bias_col = constp.tile([P, 1], f32)
            nc.sync.dma_start(out=bias_col[:, 0:1], in_=bias_d[:, None])
            xT_t = constp.tile([P, NB], i32, tag="xT")
            nc.gpsimd.memset(xT_t[:], 0)
            nc.sync.dma_start(out=xT_t[:T, :], in_=xT_d[:, :])
            sidT_t = constp.tile([P, S_TILES], i32)
            nc.sync.dma_start(out=sidT_t[:], in_=sidT_d[:, :])

            def body(iv):
                # ---- embedding bag -> qT [D, NB] ----
                qT_psum = psum1.tile([P, NB], f32, tag="qT")
                for m in range(NB):
                    etile = ebuf.tile([T, D], f32, tag="etile")
                    nc.gpsimd.indirect_dma_start(
                        out=etile[:, :],
                        out_offset=None,
                        in_=emb_d[:, :],
                        in_offset=bass.IndirectOffsetOnAxis(
                            ap=xT_t[:T, m : m + 1], axis=0
                        ),
                    )
                    nc.tensor.matmul(
                        out=qT_psum[:, m : m + 1],
                        lhsT=etile[:, :],
                        rhs=ones_col[:T, :],
                        start=True,
                        stop=True,
                    )

                # ---- L2 normalize + bias + relu, in qT layout ----
                # (walrus: at most one PSUM operand per DVE instruction)
                qT_sb = nbuf.tile([P, NB], f32, tag="qTsb")
                nc.scalar.copy(out=qT_sb[:], in_=qT_psum[:])
                sq = nbuf.tile([P, NB], f32, tag="sq")
                nc.vector.tensor_tensor(
                    out=sq[:], in0=qT_sb[:], in1=qT_sb[:],
                    op=mybir.AluOpType.mult,
                )
                ssq_psum = psum1.tile([1, NB], f32, tag="ssq")
                nc.tensor.matmul(
                    out=ssq_psum[:, :], lhsT=ones_col[:, :], rhs=sq[:, :],
                    start=True, stop=True,
                )
                std_row = nbuf.tile([1, NB], f32, tag="std")
                nc.scalar.activation(
                    out=std_row[:], in_=ssq_psum[:],
                    func=mybir.ActivationFunctionType.Sqrt,
                )
                rstd_row = nbuf.tile([1, NB], f32, tag="rstd")
                nc.vector.reciprocal(out=rstd_row[:], in_=std_row[:])
                rstd_psum = psum1.tile([P, NB], f32, tag="rstdb")
                nc.tensor.matmul(
                    out=rstd_psum[:, :], lhsT=ones_row[:, :], rhs=rstd_row[:, :],
                    start=True, stop=True,
                )
                qTn = nbuf.tile([P, NB], f32, tag="qTn")
                nc.vector.tensor_tensor(
                    out=qTn[:], in0=qT_sb[:], in1=rstd_psum[:],
                    op=mybir.AluOpType.mult,
                )
                qT = nbuf.tile([P, NB], f32, tag="qTfinal")
                nc.scalar.activation(
                    out=qT[:], in_=qTn[:],
                    func=mybir.ActivationFunctionType.Relu,
                    bias=bias_col[:, 0:1],
                )

                # ---- sampled logits, transposed: out[s, n] ----
                for t in range(S_TILES):
                    wtile = wpool.tile([P, D + 1], f32, tag="wtile")
                    nc.gpsimd.indirect_dma_start(
                        out=wtile[:, :],
                        out_offset=None,
                        in_=Wb_d[:, :],
                        in_offset=bass.IndirectOffsetOnAxis(
                            ap=sidT_t[:, t : t + 1], axis=0
                        ),
                    )
                    wT_psum = psum2.tile([P, P], f32, tag="wT")
                    nc.tensor.transpose(
                        out=wT_psum[:], in_=wtile[:, 0:D], identity=identity[:]
                    )
                    wT = wpool.tile([P, P], f32, tag="wTs")
                    nc.scalar.copy(out=wT[:], in_=wT_psum[:])
                    l_psum = psum2.tile([P, NB], f32, tag="lg")
                    nc.tensor.matmul(
                        out=l_psum[:], lhsT=wT[:], rhs=qT[:],
                        start=True, stop=True,
                    )
                    otile = opool.tile([P, NB], f32, tag="otile")
                    nc.vector.tensor_tensor(
                        out=otile[:],
                        in0=l_psum[:],
                        in1=wtile[:, D : D + 1].to_broadcast([P, NB]),
                        op=mybir.AluOpType.add,
                    )
                    nc.sync.dma_start(
                        out=out_d[t * P : (t + 1) * P, :], in_=otile[:]
                    )

            # Indirect DMA inside For_i fails this walrus build ("ISA wrong
            # length"), so benchmarking iterations are statically unrolled.
            for _ in range(iters):
                body(None)

    _fix_sync_waits(nc)
    return nc


def _build_runner(nc):
    """Jitted shard_map executor over the 8 NeuronCores (PJRT/axon path)."""
    import jax
    from jax.sharding import Mesh, PartitionSpec
    from jax.experimental.shard_map import shard_map
    from concourse import bass2jax

    bass2jax.install_neuronx_cc_hook()
    partition_name = (
        nc.partition_id_tensor.name if nc.partition_id_tensor else None
    )
    in_names, out_names, out_avals = [], [], []
    for alloc in nc.m.functions[0].allocations:
        if not isinstance(alloc, mybir.MemoryLocationSet):
            continue
        name = alloc.memorylocations[0].name
        if alloc.kind == "ExternalInput":
            if name != partition_name:
                in_names.append(name)
        elif alloc.kind == "ExternalOutput":
            out_names.append(name)
            out_avals.append(
                jax.core.ShapedArray(
                    tuple(alloc.tensor_shape), mybir.dt.np(alloc.dtype)
                )
            )
    n_params = len(in_names)
    n_outs = len(out_avals)
    all_in_names = list(in_names) + list(out_names)
    if partition_name is not None:
        all_in_names.append(partition_name)
    donate = tuple(range(n_params, n_params + n_outs))

    def _bass_body(*args):
        operands = list(args)
        if partition_name is not None:
            operands.append(bass2jax.partition_id_tensor())
        return tuple(
            bass2jax._bass_exec_p.bind(
                *operands,
                out_avals=tuple(out_avals),
                in_names=tuple(all_in_names),
                out_names=tuple(out_names),
                lowering_input_output_aliases=(),
                sim_require_finite=False,
                sim_require_nnan=False,
                nc=nc,
            )
        )

    devices = jax.devices()[:N_CORES]
    mesh = Mesh(np.asarray(devices), ("core",))
    sharded = jax.jit(
        shard_map(
            _bass_body,
            mesh=mesh,
            in_specs=(PartitionSpec("core"),) * (n_params + n_outs),
            out_specs=(PartitionSpec("core"),) * n_outs,
            check_rep=False,
        ),
        donate_argnums=donate,
        keep_unused=True,
    )

    def run(in_maps):
        import jax as _jax

        concat_in = [
            np.concatenate(
                [np.asarray(in_maps[c][k]) for c in range(N_CORES)], axis=0
            )
            for k in in_names
        ]
        concat_zeros = [
            np.zeros((N_CORES * a.shape[0], *a.shape[1:]), a.dtype)
            for a in out_avals
        ]
        out_arrs = sharded(*concat_in, *concat_zeros)
        _jax.block_until_ready(out_arrs)
        return [
            {
                k: np.asarray(out_arrs[i]).reshape(
                    N_CORES, *out_avals[i].shape
                )[c]
                for i, k in enumerate(out_names)
            }
            for c in range(N_CORES)
        ]

    return run


_runner_cache = {}


def _get_runner(iters: int = 1):
    if iters not in _runner_cache:
        _runner_cache[iters] = _build_runner(build_nc(iters))
    return _runner_cache[iters]


def _prep_in_maps(x, sample_ids, emb_table, bias, W, b_cls):
    x = np.asarray(x)
    sample_ids = np.asarray(sample_ids)
    emb_table = np.ascontiguousarray(np.asarray(emb_table, dtype=np.float32))
    bias = np.ascontiguousarray(np.asarray(bias, dtype=np.float32))
    Wb = np.concatenate(
        [
            np.asarray(W, dtype=np.float32),
            np.asarray(b_cls, dtype=np.float32)[:, None],
        ],
        axis=1,
    )
    sid = np.zeros((S_PAD,), dtype=np.int32)
    sid[:S] = sample_ids.astype(np.int32)
    sidT = np.ascontiguousarray(sid.reshape(S_TILES, P).T)  # [P, S_TILES]
    in_maps = []
    for c in range(N_CORES):
        xc = x[c * NB : (c + 1) * NB].astype(np.int32)       # [NB, T]
        in_maps.append(
            {
                "xT": np.ascontiguousarray(xc.T),            # [T, NB]
                "emb_table": emb_table,
                "bias": bias,
                "Wb": Wb,
                "sidT": sidT,
            }
        )
    return in_maps


def kernel(x, sample_ids, emb_table, bias, W, b_cls):
    run = _get_runner(1)
    in_maps = _prep_in_maps(x, sample_ids, emb_table, bias, W, b_cls)
    results = run(in_maps)
    out = np.empty((N, S), dtype=np.float32)
    for c in range(N_CORES):
        out[c * NB : (c + 1) * NB, :] = results[c]["out"][:S, :].T
    return out



# revision 1
# speedup vs baseline: 1.7430x; 1.7430x over previous
"""Trainium2 Bass kernel for nn_Net_48498770706963 (retrieval_knn).

Computation (see reference):
  emb   = sum_t emb_table[x[n, t]]          # embedding-bag over T=100 tokens
  query = relu(emb / ||emb||_2 + bias)      # [N, D]
  out   = query @ W[sample_ids].T + b_cls[sample_ids]   # [N, S]

Distribution: data-parallel over the batch. Each of the 8 NeuronCores
computes the embedding-bag + normalization for its 128 samples and the full
sampled-logits panel [S, 128] for those samples (class weight table
replicated). No collectives. Host work is limited to slicing/padding/
transposing inputs and reassembling the output.

Per-core kernel pipeline (all validated on silicon):
  - 128 indirect DMA gathers [100, 128] (one per sample) feed per-sample
    ones-matmuls that reduce tokens on the TensorEngine directly into the
    transposed query layout qT [D, 128].
  - L2-normalization in transposed layout: DVE square, ones-matmul column
    sum, ACT sqrt + DVE reciprocal, ones-matmul partition broadcast, DVE
    scale, ACT relu with per-partition bias.
  - 157 indirect DMA gathers of [128, 129] rows from the packed [W | b_cls]
    table, PE transpose of the W block, logits matmul against qT, DVE
    bias-add (broadcast of the gathered bias column) fused with the
    PSUM->SBUF copy, direct DMA store of the [128, 128] output tile.
"""

import numpy as np

import concourse.bass as bass
import concourse.mybir as mybir
from concourse.tile import TileContext
from concourse.masks import make_identity

N, T, D = 1024, 100, 128
S = 20000
V_IN = 135909
V_OUT = 670091
N_CORES = 8
P = 128
NB = N // N_CORES            # 128 samples per core
S_TILES = (S + P - 1) // P   # 157
S_PAD = S_TILES * P          # 20096

f32 = mybir.dt.float32
i32 = mybir.dt.int32

_MAX_WAITS = 1


def _fix_sync_waits(nc):
    """This walrus build rejects instructions carrying more than one sem
    wait ('Too many sync wait commands'). Hoist excess waits onto NoOps
    inserted immediately before, on the same engine stream."""
    for fn in nc.m.functions:
        for bb in fn.blocks:
            out = []
            changed = False
            for inst in bb.instructions:
                si = inst.sync_info
                waits = list(si.on_wait) if si is not None else []
                if len(waits) > _MAX_WAITS:
                    changed = True
                    excess, keep = waits[:-_MAX_WAITS], waits[-_MAX_WAITS:]
                    for k in range(0, len(excess), _MAX_WAITS):
                        nop = mybir.InstNoOp(
                            name=nc.get_next_instruction_name(), ins=[], outs=[]
                        )
                        nop.engine = inst.engine
                        nop.sync_info = mybir.SyncInfo(
                            on_wait=excess[k : k + _MAX_WAITS], on_update=[]
                        )
                        out.append(nop)
                    si.on_wait = keep
                out.append(inst)
            if changed:
                bb.instructions = out


def build_nc(iters: int = 1):
    """Build the per-core Bass program. iters>1 wraps the body in a For_i
    loop (used only for wall-clock benchmarking in test.py)."""
    nc = bass.Bass()
    xT_d = nc.declare_dram_parameter("xT", [T, NB], i32, isOutput=False)
    emb_d = nc.declare_dram_parameter(
        "emb_table", [V_IN + 1, D], f32, isOutput=False
    )
    bias_d = nc.declare_dram_parameter("bias", [D], f32, isOutput=False)
    Wb_d = nc.declare_dram_parameter("Wb", [V_OUT, D + 1], f32, isOutput=False)
    sidT_d = nc.declare_dram_parameter("sidT", [P, S_TILES], i32, isOutput=False)
    out_d = nc.declare_dram_parameter("out", [S_PAD, NB], f32, isOutput=True)

    with TileContext(nc) as tc:
        with (
            tc.tile_pool(name="const", bufs=1) as constp,
            tc.tile_pool(name="ebuf", bufs=8) as ebuf,
            tc.tile_pool(name="nbuf", bufs=2) as nbuf,
            tc.tile_pool(name="psum1", bufs=1, space="PSUM") as psum1,
            tc.tile_pool(name="psum2", bufs=2, space="PSUM") as psum2,
            tc.tile_pool(name="wpool", bufs=6) as wpool,
            tc.tile_pool(name="opool", bufs=6) as opool,
        ):
            # ---- constants ----
            identity = constp.tile([P, P], f32)
            make_identity(nc, identity[:])
            ones_col = constp.tile([P, 1], f32)
            nc.vector.memset(ones_col[:], 1.0)
            ones_row = constp.tile([1, P], f32)
            nc.vector.memset(ones_row[:], 1.0)
            bias_col = constp.tile([P, 1], f32)
            nc.sync.dma_start(out=bias_col[:, 0:1], in_=bias_d[:, None])
            xT_t = constp.tile([P, NB], i32, tag="xT")
            nc.gpsimd.memset(xT_t[:], 0)
            nc.sync.dma_start(out=xT_t[:T, :], in_=xT_d[:, :])
            sidT_t = constp.tile([P, S_TILES], i32)
            nc.sync.dma_start(out=sidT_t[:], in_=sidT_d[:, :])

            def body(iv):
                # ---- embedding bag -> qT [D, NB] ----
                qT_psum = psum1.tile([P, NB], f32, tag="qT")
                for m in range(NB):
                    etile = ebuf.tile([T, D], f32, tag="etile")
                    nc.gpsimd.indirect_dma_start(
                        out=etile[:, :],
                        out_offset=None,
                        in_=emb_d[:, :],
                        in_offset=bass.IndirectOffsetOnAxis(
                            ap=xT_t[:T, m : m + 1], axis=0
                        ),
                    )
                    nc.tensor.matmul(
                        out=qT_psum[:, m : m + 1],
                        lhsT=etile[:, :],
                        rhs=ones_col[:T, :],
                        start=True,
                        stop=True,
                    )

                # ---- L2 normalize + bias + relu, in qT layout ----
                # (walrus: at most one PSUM operand per DVE instruction)
                qT_sb = nbuf.tile([P, NB], f32, tag="qTsb")
                nc.scalar.copy(out=qT_sb[:], in_=qT_psum[:])
                sq = nbuf.tile([P, NB], f32, tag="sq")
                nc.vector.tensor_tensor(
                    out=sq[:], in0=qT_sb[:], in1=qT_sb[:],
                    op=mybir.AluOpType.mult,
                )
                ssq_psum = psum1.tile([1, NB], f32, tag="ssq")
                nc.tensor.matmul(
                    out=ssq_psum[:, :], lhsT=ones_col[:, :], rhs=sq[:, :],
                    start=True, stop=True,
                )
                std_row = nbuf.tile([1, NB], f32, tag="std")
                nc.scalar.activation(
                    out=std_row[:], in_=ssq_psum[:],
                    func=mybir.ActivationFunctionType.Sqrt,
                )
                rstd_row = nbuf.tile([1, NB], f32, tag="rstd")
                nc.vector.reciprocal(out=rstd_row[:], in_=std_row[:])
                rstd_psum = psum1.tile([P, NB], f32, tag="rstdb")
                nc.tensor.matmul(
                    out=rstd_psum[:, :], lhsT=ones_row[:, :], rhs=rstd_row[:, :],
                    start=True, stop=True,
                )
                qTn = nbuf.tile([P, NB], f32, tag="qTn")
                nc.vector.tensor_tensor(
                    out=qTn[:], in0=qT_sb[:], in1=rstd_psum[:],
                    op=mybir.AluOpType.mult,
                )
                qT = nbuf.tile([P, NB], f32, tag="qTfinal")
                nc.scalar.activation(
                    out=qT[:], in_=qTn[:],
                    func=mybir.ActivationFunctionType.Relu,
                    bias=bias_col[:, 0:1],
                )

                # ---- sampled logits, transposed: out[s, n] ----
                for t in range(S_TILES):
                    wtile = wpool.tile([P, D + 1], f32, tag="wtile")
                    nc.gpsimd.indirect_dma_start(
                        out=wtile[:, :],
                        out_offset=None,
                        in_=Wb_d[:, :],
                        in_offset=bass.IndirectOffsetOnAxis(
                            ap=sidT_t[:, t : t + 1], axis=0
                        ),
                    )
                    wT_psum = psum2.tile([P, P], f32, tag="wT")
                    nc.tensor.transpose(
                        out=wT_psum[:], in_=wtile[:, 0:D], identity=identity[:]
                    )
                    wT = wpool.tile([P, P], f32, tag="wTs")
                    nc.scalar.copy(out=wT[:], in_=wT_psum[:])
                    l_psum = psum2.tile([P, NB], f32, tag="lg")
                    nc.tensor.matmul(
                        out=l_psum[:], lhsT=wT[:], rhs=qT[:],
                        start=True, stop=True,
                    )
                    otile = opool.tile([P, NB], f32, tag="otile")
                    nc.vector.tensor_tensor(
                        out=otile[:],
                        in0=l_psum[:],
                        in1=wtile[:, D : D + 1].to_broadcast([P, NB]),
                        op=mybir.AluOpType.add,
                    )
                    nc.sync.dma_start(
                        out=out_d[t * P : (t + 1) * P, :], in_=otile[:]
                    )

            # Indirect DMA inside For_i fails this walrus build ("ISA wrong
            # length"), so benchmarking iterations are statically unrolled.
            for _ in range(iters):
                body(None)

    _fix_sync_waits(nc)
    return nc


def _build_runner(nc):
    """Jitted shard_map executor over the 8 NeuronCores (PJRT/axon path)."""
    import jax
    from jax.sharding import Mesh, PartitionSpec
    from jax.experimental.shard_map import shard_map
    from concourse import bass2jax

    bass2jax.install_neuronx_cc_hook()
    partition_name = (
        nc.partition_id_tensor.name if nc.partition_id_tensor else None
    )
    in_names, out_names, out_avals = [], [], []
    for alloc in nc.m.functions[0].allocations:
        if not isinstance(alloc, mybir.MemoryLocationSet):
            continue
        name = alloc.memorylocations[0].name
        if alloc.kind == "ExternalInput":
            if name != partition_name:
                in_names.append(name)
        elif alloc.kind == "ExternalOutput":
            out_names.append(name)
            out_avals.append(
                jax.core.ShapedArray(
                    tuple(alloc.tensor_shape), mybir.dt.np(alloc.dtype)
                )
            )
    n_params = len(in_names)
    n_outs = len(out_avals)
    all_in_names = list(in_names) + list(out_names)
    if partition_name is not None:
        all_in_names.append(partition_name)
    donate = tuple(range(n_params, n_params + n_outs))

    def _bass_body(*args):
        operands = list(args)
        if partition_name is not None:
            operands.append(bass2jax.partition_id_tensor())
        return tuple(
            bass2jax._bass_exec_p.bind(
                *operands,
                out_avals=tuple(out_avals),
                in_names=tuple(all_in_names),
                out_names=tuple(out_names),
                lowering_input_output_aliases=(),
                sim_require_finite=False,
                sim_require_nnan=False,
                nc=nc,
            )
        )

    devices = jax.devices()[:N_CORES]
    mesh = Mesh(np.asarray(devices), ("core",))
    sharded = jax.jit(
        shard_map(
            _bass_body,
            mesh=mesh,
            in_specs=(PartitionSpec("core"),) * (n_params + n_outs),
            out_specs=(PartitionSpec("core"),) * n_outs,
            check_rep=False,
        ),
        donate_argnums=donate,
        keep_unused=True,
    )

    def run(in_maps):
        import jax as _jax

        concat_in = [
            np.concatenate(
                [np.asarray(in_maps[c][k]) for c in range(N_CORES)], axis=0
            )
            for k in in_names
        ]
        concat_zeros = [
            np.zeros((N_CORES * a.shape[0], *a.shape[1:]), a.dtype)
            for a in out_avals
        ]
        out_arrs = sharded(*concat_in, *concat_zeros)
        _jax.block_until_ready(out_arrs)
        return [
            {
                k: np.asarray(out_arrs[i]).reshape(
                    N_CORES, *out_avals[i].shape
                )[c]
                for i, k in enumerate(out_names)
            }
            for c in range(N_CORES)
        ]

    return run


_runner_cache = {}


def _get_runner(iters: int = 1):
    if iters not in _runner_cache:
        _runner_cache[iters] = _build_runner(build_nc(iters))
    return _runner_cache[iters]


def _prep_in_maps(x, sample_ids, emb_table, bias, W, b_cls):
    x = np.asarray(x)
    sample_ids = np.asarray(sample_ids)
    emb_table = np.ascontiguousarray(np.asarray(emb_table, dtype=np.float32))
    bias = np.ascontiguousarray(np.asarray(bias, dtype=np.float32))
    Wb = np.concatenate(
        [
            np.asarray(W, dtype=np.float32),
            np.asarray(b_cls, dtype=np.float32)[:, None],
        ],
        axis=1,
    )
    sid = np.zeros((S_PAD,), dtype=np.int32)
    sid[:S] = sample_ids.astype(np.int32)
    sidT = np.ascontiguousarray(sid.reshape(S_TILES, P).T)  # [P, S_TILES]
    in_maps = []
    for c in range(N_CORES):
        xc = x[c * NB : (c + 1) * NB].astype(np.int32)       # [NB, T]
        in_maps.append(
            {
                "xT": np.ascontiguousarray(xc.T),            # [T, NB]
                "emb_table": emb_table,
                "bias": bias,
                "Wb": Wb,
                "sidT": sidT,
            }
        )
    return in_maps


def kernel(x, sample_ids, emb_table, bias, W, b_cls):
    run = _get_runner(1)
    in_maps = _prep_in_maps(x, sample_ids, emb_table, bias, W, b_cls)
    results = run(in_maps)
    out = np.empty((N, S), dtype=np.float32)
    for c in range(N_CORES):
        out[c * NB : (c + 1) * NB, :] = results[c]["out"][:S, :].T
    return out

